# revision 19
# baseline (speedup 1.0000x reference)
"""Trainium2 Bass kernel for nn_AutoEncoder (ragged_sequence).

Math: reference only reads the linear-attention cumsum state at t = n_b - 1
(the last valid row of each graph), so the [B,T,F,F] cumsum collapses to a
per-graph S_b = K_b^T V_b (FxF), Z_b = sum_r mask_r K_b[r], q_b = Q at the
last valid row:  z_b = (q_b @ S_b) / (q_b . Z_b + 1e-5), then the MLP.

Sharding: batch dim B=64 -> 8 graphs per core across 8 NeuronCores (data
parallel); K/Q/V/MLP weights replicated. Padding (ragged case) is
right-aligned on host so the last valid row is always row T-1.
"""

import numpy as np

import concourse.bass as bass
import concourse.bacc as bacc_mod
import concourse.mybir as mybir
from concourse.bass_utils import run_bass_kernel_spmd
from concourse.tile import TileContext

F32 = mybir.dt.float32
BF16 = mybir.dt.bfloat16
AF = mybir.ActivationFunctionType
ALU = mybir.AluOpType
AX = mybir.AxisListType

B, T, D, F, NCLS = 64, 256, 256, 128, 257
NCORES = 8
GPC = B // NCORES          # graphs per core
TPG = T // 128             # row-tiles per graph (2)
NT = GPC * TPG             # row-tiles per core (16)
DB = D // 128              # d-blocks (2)

TRACE = False              # test.py flips this to collect a profile
LAST_RESULT = None         # BassKernelResults of the most recent run
LAST_IN_MAPS = None        # per-core input maps of the most recent run

_NC_CACHE = {}


def _build(add_bias: bool, mask_v: bool) -> bass.Bass:
    nc = bacc_mod.Bacc()

    y = nc.dram_tensor("y", [NT * 128, D], F32, kind="ExternalInput")
    mask = nc.dram_tensor("mask", [128, NT], F32, kind="ExternalInput")
    wk = nc.dram_tensor("wk", [D, F], BF16, kind="ExternalInput")
    wq = nc.dram_tensor("wq", [D, F], BF16, kind="ExternalInput")
    wv = nc.dram_tensor("wv", [D, F], BF16, kind="ExternalInput")
    w1 = nc.dram_tensor("w1", [F, F], F32, kind="ExternalInput")
    b1 = nc.dram_tensor("b1", [1, F], F32, kind="ExternalInput")
    w2 = nc.dram_tensor("w2", [F, NCLS], F32, kind="ExternalInput")
    b2 = nc.dram_tensor("b2", [1, NCLS], F32, kind="ExternalInput")
    ident = nc.dram_tensor("ident", [128, 128], F32, kind="ExternalInput")
    ones = nc.dram_tensor("ones", [1, 128], F32, kind="ExternalInput")
    # packed per-graph bias rows: [kk; vv; qq] each [1, F] (only if add_bias)
    kvq_b = nc.dram_tensor("kvq_b", [3, F], F32, kind="ExternalInput")

    z_out = nc.dram_tensor("z_out", [GPC, F], F32, kind="ExternalOutput")
    lg_out = nc.dram_tensor("lg_out", [GPC, NCLS], F32, kind="ExternalOutput")

    with TileContext(nc) as tc:
        with (
            tc.tile_pool(name="const", bufs=1) as cpool,
            tc.tile_pool(name="xin", bufs=3) as xpool,
            tc.tile_pool(name="work", bufs=3) as wpool,
            tc.tile_pool(name="stat", bufs=3) as spool,
            tc.tile_pool(name="fin", bufs=1) as fpool,
            tc.tile_pool(name="psT", bufs=1, space="PSUM") as psT,
            tc.tile_pool(name="psKV", bufs=2, space="PSUM") as psKV,
            tc.tile_pool(name="psS", bufs=2, space="PSUM") as psS,
            tc.tile_pool(name="psZQ", bufs=2, space="PSUM") as psZQ,
            tc.tile_pool(name="psND", bufs=1, space="PSUM") as psND,
        ):
            # ---- constants / weights ----
            # first x loads go before the weight DMAs (queue order = issue
            # order; stats work can then start at ~1.5us instead of ~7us)
            x_pre = []
            for g in range(2):
                x_g = xpool.tile([128, TPG, D], F32, tag="x")
                nc.sync.dma_start(
                    out=x_g[:],
                    in_=y[g * T:(g + 1) * T, :].rearrange(
                        "(t p) d -> p t d", p=128
                    ),
                )
                x_pre.append(x_g)
            wkv_sb = cpool.tile([128, DB, 2, F], BF16, tag="wkv")
            nc.sync.dma_start(
                out=wkv_sb[:, :, 0, :], in_=wk.rearrange("(b p) f -> p b f", p=128)
            )
            nc.sync.dma_start(
                out=wkv_sb[:, :, 1, :], in_=wv.rearrange("(b p) f -> p b f", p=128)
            )
            wq_sb = cpool.tile([128, DB, F], BF16, tag="wq")
            nc.sync.dma_start(
                out=wq_sb[:], in_=wq.rearrange("(b p) f -> p b f", p=128)
            )
            id_sb = cpool.tile([128, 128], F32, tag="id")
            nc.sync.dma_start(out=id_sb[:], in_=ident[:])
            ones_sb = cpool.tile([1, 128], F32, tag="ones")
            nc.sync.dma_start(out=ones_sb[:], in_=ones[:])
            m_sb = cpool.tile([128, NT], F32, tag="mask")
            nc.sync.dma_start(out=m_sb[:], in_=mask[:])
            w1_sb = cpool.tile([128, F], F32, tag="w1")
            nc.sync.dma_start(out=w1_sb[:], in_=w1[:])
            w2_sb = cpool.tile([128, NCLS], F32, tag="w2")
            nc.sync.dma_start(out=w2_sb[:], in_=w2[:])
            b1_sb = cpool.tile([1, F], F32, tag="b1")
            nc.sync.dma_start(out=b1_sb[:], in_=b1[:])
            b2_sb = cpool.tile([1, NCLS], F32, tag="b2")
            nc.sync.dma_start(out=b2_sb[:], in_=b2[:])
            if add_bias:
                kvq_sb = cpool.tile([1, 3, F], F32, tag="kvq")
                for i in range(3):
                    nc.sync.dma_start(out=kvq_sb[0:1, i, :], in_=kvq_b[i:i + 1, :])
            eps_sb = cpool.tile([128, 1], F32, tag="eps")
            nc.vector.memset(eps_sb[:], 1e-5)

            # numT cols 0..GPC-1, den at row 0 cols GPC..2*GPC-1
            nd_ps = psND.tile([128, 2 * GPC], F32, tag="nd")

            for g in range(GPC):
                s_ps = psS.tile([128, F], F32, tag="s")
                zq_ps = psZQ.tile([128, 2], F32, tag="zq")

                # ---- whole-graph load: [row_in_tile, t2, d] ----
                if g < 2:
                    x_g = x_pre[g]
                else:
                    x_g = xpool.tile([128, TPG, D], F32, tag="x")
                    nc.sync.dma_start(
                        out=x_g[:],
                        in_=y[g * T:(g + 1) * T, :].rearrange(
                            "(t p) d -> p t d", p=128
                        ),
                    )

                # ---- LayerNorm stats for both tiles at once ----
                xsq = wpool.tile([128, TPG, D], F32, tag="xsq")
                ssqa = spool.tile([128, 1], F32, tag="ssqa")
                ssqb = spool.tile([128, 1], F32, tag="ssqb")
                nc.scalar.activation(xsq[:, 0, :], x_g[:, 0, :], AF.Square,
                                     accum_out=ssqa[:])
                nc.scalar.activation(xsq[:, 1, :], x_g[:, 1, :], AF.Square,
                                     accum_out=ssqb[:])
                sx = spool.tile([128, TPG], F32, tag="sx")
                nc.vector.tensor_reduce(sx[:], x_g[:], axis=AX.X, op=ALU.add)
                negmu = spool.tile([128, TPG], F32, tag="negmu")
                nc.vector.tensor_scalar(negmu[:], sx[:], -1.0 / D, None, ALU.mult)
                var = spool.tile([128, TPG], F32, tag="var")
                nc.vector.tensor_scalar(var[:, 0:1], ssqa[:], 1.0 / D, 1e-5, ALU.mult, ALU.add)
                nc.vector.tensor_scalar(var[:, 1:2], ssqb[:], 1.0 / D, 1e-5, ALU.mult, ALU.add)
                musq = spool.tile([128, TPG], F32, tag="musq")
                nc.vector.tensor_tensor(musq[:], negmu[:], negmu[:], ALU.mult)
                nc.vector.tensor_tensor(var[:], var[:], musq[:], ALU.subtract)
                I32 = mybir.dt.int32
                yb = spool.tile([128, TPG], I32, tag="yb")
                nc.vector.tensor_scalar(
                    yb[:], var[:].bitcast(I32), 1, -1,
                    ALU.arith_shift_right, ALU.bitwise_xor,
                )
                nc.vector.tensor_scalar(yb[:], yb[:], 0x5F3759E0, None, ALU.add)
                rstd = yb[:].bitcast(F32)
                nt = spool.tile([128, TPG], F32, tag="nt")
                for _ in range(2):
                    nc.vector.tensor_tensor(nt[:], rstd, rstd, ALU.mult)
                    nc.vector.tensor_tensor(nt[:], nt[:], var[:], ALU.mult)
                    nc.vector.tensor_scalar(nt[:], nt[:], -0.5, 1.5, ALU.mult, ALU.add)
                    nc.vector.tensor_tensor(rstd, rstd, nt[:], ALU.mult)

                # ---- normalize + transpose (per row-tile) ----
                xn_g = wpool.tile([128, TPG, D], BF16, tag="xn")
                xnTs = []
                for t2 in range(TPG):
                    nc.vector.tensor_scalar(
                        xn_g[:, t2, :], x_g[:, t2, :],
                        negmu[:, t2:t2 + 1], yb[:, t2:t2 + 1].bitcast(F32),
                        ALU.add, ALU.mult,
                    )
                    xnT = wpool.tile([128, DB, 128], BF16, tag="xnT")
                    for db in range(DB):
                        nc.sync.dma_start_transpose(
                            xnT[:, db, :], xn_g[:, t2, db * 128:(db + 1) * 128]
                        )
                    xnTs.append(xnT)

                # ---- [K|V] = . @ [wk|wv] for both tiles (one bank) ----
                kvp = psKV.tile([128, TPG, 2, F], F32, tag="kv")
                kp = kvp[:, :, 0, :]
                vp = kvp[:, :, 1, :]
                for t2 in range(TPG):
                    for db in range(DB):
                        nc.tensor.matmul(
                            kvp[:, t2, :, :], lhsT=xnTs[t2][:, db, :],
                            rhs=wkv_sb[:, db, :, :],
                            start=(db == 0), stop=(db == DB - 1 and not add_bias),
                        )
                    if add_bias:
                        nc.tensor.matmul(
                            kvp[:, t2, :, :], lhsT=ones_sb[0:1, :],
                            rhs=kvq_sb[0:1, 0:2, :],
                            start=False, stop=True,
                        )

                # phi(v) = elu(v)+1 = max(min(exp(v), 1), v + 1), both tiles
                e_t = wpool.tile([128, TPG, F], F32, tag="e")
                nc.scalar.activation(e_t[:], kp, AF.Exp)
                p1 = wpool.tile([128, TPG, F], F32, tag="p1")
                nc.vector.tensor_scalar(p1[:], kp, 1.0, None, ALU.add)
                nc.vector.tensor_scalar(e_t[:], e_t[:], 1.0, None, ALU.min)
                k_t = wpool.tile([128, TPG, F], F32, tag="k")
                nc.vector.tensor_tensor(k_t[:], e_t[:], p1[:], ALU.max)
                v_t = wpool.tile([128, TPG, F], F32, tag="v")
                if mask_v:
                    nc.vector.tensor_scalar(
                        v_t[:, 0, :], vp[:, 0:1, :],
                        m_sb[:, g * TPG:g * TPG + 1], None, ALU.mult
                    )
                    nc.vector.tensor_scalar(
                        v_t[:, 1, :], vp[:, 1:2, :],
                        m_sb[:, g * TPG + 1:g * TPG + 2], None, ALU.mult
                    )
                else:
                    nc.scalar.copy(v_t[:], vp)

                # ---- per-graph accumulations ----
                for t2 in range(TPG):
                    t = g * TPG + t2
                    nc.tensor.matmul(
                        s_ps[:], lhsT=k_t[:, t2, :], rhs=v_t[:, t2, :],
                        start=(t2 == 0), stop=(t2 == TPG - 1),
                    )
                    nc.tensor.matmul(
                        zq_ps[:, 0:1], lhsT=k_t[:, t2, :], rhs=m_sb[:, t:t + 1],
                        start=(t2 == 0), stop=(t2 == TPG - 1),
                    )
                for db in range(DB):
                    nc.tensor.matmul(
                        zq_ps[:, 1:2], lhsT=wq_sb[:, db, :],
                        rhs=xnTs[TPG - 1][:, db, 127:128],
                        start=(db == 0), stop=(db == DB - 1 and not add_bias),
                    )
                if add_bias:
                    nc.tensor.matmul(
                        zq_ps[:, 1:2], lhsT=kvq_sb[0:1, 2, :],
                        rhs=ones_sb[0:1, 0:1],
                        start=False, stop=True,
                    )

                # ---- finalize graph g ----
                eq = spool.tile([128, 1], F32, tag="eq")
                nc.scalar.activation(eq[:], zq_ps[:, 1:2], AF.Exp)
                p1q = spool.tile([128, 1], F32, tag="p1q")
                nc.vector.tensor_scalar(p1q[:], zq_ps[:, 1:2], 1.0, None, ALU.add)
                nc.vector.tensor_scalar(eq[:], eq[:], 1.0, None, ALU.min)
                q_sb = spool.tile([128, 1], F32, tag="q")
                nc.vector.tensor_tensor(q_sb[:], eq[:], p1q[:], ALU.max)
                zt_sb = spool.tile([128, 1], F32, tag="zt")
                nc.scalar.copy(zt_sb[:], zq_ps[:, 0:1])
                s_sb = wpool.tile([128, F], F32, tag="ssb")
                nc.scalar.copy(s_sb[:], s_ps[:])
                # numT[:, g] = S^T q ; den[g] = Z . q
                nc.tensor.matmul(
                    nd_ps[:, g:g + 1], lhsT=s_sb[:], rhs=q_sb[:],
                    start=True, stop=True,
                )
                nc.tensor.matmul(
                    nd_ps[0:1, GPC + g:GPC + g + 1], lhsT=zt_sb[:], rhs=q_sb[:],
                    start=True, stop=True,
                )

            # ---- z = num/(den+1e-5); fold 1/den into post-matmul scales ----
            numT_sb = fpool.tile([128, GPC], F32, tag="numT")
            nc.scalar.copy(numT_sb[:], nd_ps[:, 0:GPC])
            den_sb = fpool.tile([1, GPC], F32, tag="den")
            nc.scalar.activation(
                den_sb[:], nd_ps[0:1, GPC:2 * GPC], AF.Copy, bias=1e-5
            )
            d8_ps = psT.tile([GPC, 128], F32, tag="tp")
            nc.tensor.transpose(d8_ps[:, 0:1], den_sb[:], id_sb[0:1, 0:1])
            den8 = fpool.tile([GPC, 1], F32, tag="den8")
            nc.scalar.copy(den8[:], d8_ps[:, 0:1])
            rden8 = fpool.tile([GPC, 1], F32, tag="rden8")
            nc.vector.reciprocal(rden8[:], den8[:])

            zz_ps = psT.tile([GPC, 128], F32, tag="tp")
            nc.tensor.transpose(zz_ps[:], numT_sb[:], id_sb[:])
            z_sb = fpool.tile([GPC, F], F32, tag="z")
            nc.scalar.activation(z_sb[:], zz_ps[:], AF.Copy, scale=rden8[:])
            nc.sync.dma_start(out=z_out[:], in_=z_sb[:])

            # ---- MLP: h = tanh((num @ W1 + den*b1) / den) = tanh(z@W1+b1) ----
            h_ps = psND.tile([GPC, NCLS], F32, tag="nd")
            nc.tensor.matmul(
                h_ps[:, 0:F], lhsT=numT_sb[:], rhs=w1_sb[:], start=True, stop=False
            )
            nc.tensor.matmul(
                h_ps[:, 0:F], lhsT=den_sb[:], rhs=b1_sb[:],
                start=False, stop=True,
            )
            h_sb = fpool.tile([GPC, F], F32, tag="h")
            nc.scalar.activation(h_sb[:], h_ps[:, 0:F], AF.Tanh, scale=rden8[:])
            hT_ps = psT.tile([128, GPC], F32, tag="tp")
            nc.tensor.transpose(hT_ps[:], h_sb[:], id_sb[0:GPC, 0:GPC])
            hT_sb = fpool.tile([128, GPC], F32, tag="hT")
            nc.scalar.copy(hT_sb[:], hT_ps[:])
            lg_ps = psND.tile([GPC, NCLS], F32, tag="nd")
            nc.tensor.matmul(
                lg_ps[:], lhsT=hT_sb[:], rhs=w2_sb[:], start=True, stop=False
            )
            nc.tensor.matmul(
                lg_ps[:], lhsT=ones_sb[0:1, 0:GPC], rhs=b2_sb[:],
                start=False, stop=True,
            )
            lg_sb = fpool.tile([GPC, NCLS], F32, tag="lg")
            nc.scalar.copy(lg_sb[:], lg_ps[:])
            nc.sync.dma_start(out=lg_out[:], in_=lg_sb[:])

    return nc


def _prep_host(x, batch, ln_g, ln_b, Wk, Wq, Wv):
    """Right-align each graph's rows into a padded [B, T, D] + valid mask."""
    n = np.bincount(batch.astype(np.int64), minlength=B).astype(np.int64)
    if n.sum() != x.shape[0]:
        raise ValueError("batch ids out of range")
    if np.all(n == T):
        y = np.ascontiguousarray(x.reshape(B, T, D))
        m = np.ones((B, T), np.float32)
    else:
        ptr = np.zeros(B + 1, np.int64)
        np.cumsum(n, out=ptr[1:])
        y = np.zeros((B, T, D), np.float32)
        m = np.zeros((B, T), np.float32)
        for b in range(B):
            k = min(int(n[b]), T)
            if k:
                y[b, T - k:] = x[ptr[b]:ptr[b] + k]
                m[b, T - k:] = 1.0
    wkp = np.ascontiguousarray(ln_g[:, None] * Wk)
    wqp = np.ascontiguousarray(ln_g[:, None] * Wq)
    wvp = np.ascontiguousarray(ln_g[:, None] * Wv)
    kvq = np.stack([ln_b @ Wk, ln_b @ Wv, ln_b @ Wq]).astype(np.float32)
    return y, m, wkp, wqp, wvp, kvq


def kernel(x, batch, Wk, Wq, Wv, ln_g, ln_b, W1, b1, W2, b2):
    global LAST_RESULT
    x = np.asarray(x, np.float32)
    batch = np.asarray(batch)
    y, m, wkp, wqp, wvp, kvq = _prep_host(
        x, batch, np.asarray(ln_g, np.float32), np.asarray(ln_b, np.float32),
        np.asarray(Wk, np.float32), np.asarray(Wq, np.float32),
        np.asarray(Wv, np.float32),
    )
    add_bias = bool(np.any(np.asarray(ln_b) != 0.0))
    ragged = not bool(np.all(m == 1.0))
    key = (add_bias, add_bias and ragged)
    if key not in _NC_CACHE:
        nc_new = _build(*key)
        if not nc_new.is_finalized():
            nc_new.finalize()
        _NC_CACHE[key] = nc_new
    nc = _NC_CACHE[key]

    import ml_dtypes
    shared = {
        "wk": wkp.astype(ml_dtypes.bfloat16),
        "wq": wqp.astype(ml_dtypes.bfloat16),
        "wv": wvp.astype(ml_dtypes.bfloat16),
        "w1": np.ascontiguousarray(W1, np.float32),
        "b1": np.asarray(b1, np.float32).reshape(1, F),
        "w2": np.ascontiguousarray(W2, np.float32),
        "b2": np.asarray(b2, np.float32).reshape(1, NCLS),
        "ident": np.eye(128, dtype=np.float32),
        "ones": np.ones((1, 128), np.float32),
        "kvq_b": kvq,
    }
    in_maps = []
    for c in range(NCORES):
        yc = np.ascontiguousarray(
            y[c * GPC:(c + 1) * GPC].reshape(NT * 128, D)
        )
        mc = np.ascontiguousarray(
            m[c * GPC:(c + 1) * GPC].reshape(NT, 128).T
        )
        in_maps.append({"y": yc, "mask": mc, **shared})

    want_trace = TRACE
    if want_trace:
        try:
            from antenv.axon_hooks import get_axon_ntff_profile_hook  # noqa: F401
        except ImportError:
            want_trace = False
    res = run_bass_kernel_spmd(
        nc, in_maps, list(range(NCORES)), trace=want_trace,
    )
    LAST_RESULT = res
    globals()["LAST_IN_MAPS"] = in_maps
    z = np.concatenate([res.results[c]["z_out"] for c in range(NCORES)], 0)
    lg = np.concatenate([res.results[c]["lg_out"] for c in range(NCORES)], 0)
    n_pred = np.argmax(lg, axis=1).astype(np.int32)
    return z, lg, n_pred
